# revision 8
# baseline (speedup 1.0000x reference)
"""Trainium2 Bass kernel for nn_MoE10DimDecoder (B=2,T=2048,V=32000,D=768,L=6, top-3-of-10 MoE).

Sharding (8 cores): core c = (g=c//4 batch, m=c%4).
 - Trunk token-sharded: core owns tokens [512m, 512m+512) of batch g (residual xT
   kept on-chip in [d, t] layout).  FFN/MoE/norms are rowwise -> no collectives.
 - Attention head-sharded inside each 4-core group: member m computes q-heads
   3m..3m+2 / kv-head m over the FULL 2048 tokens (AllGather of normed h), then
   the wo partial product is ReduceScatter'ed back to token shards.  This keeps
   causal work balanced and the SPMD program identical on all cores.
 - MoE: routing depends only on consciousness_states -> computed on host; each
   group's 3 selected experts' weights are gathered/folded on host per layer and
   streamed as dense FFNs (tw folded into the down projection).
 - Tied vocab heads sharded by vocab: core owns a 4000-wide slice of both heads.

Matmul dtypes: fp32r (TF32-like, full PE rate at N>=256) for FFN/MoE/logit heads;
bf16 for attention internals.  All layernorm gains folded into adjacent weights
on the host; rms rsqrt factors exact fp32 on device.
"""
import numpy as np

import concourse.bass as bass
import concourse.mybir as mybir
import concourse.tile as tile
import concourse.bacc as bacc
from concourse import masks
from concourse.bass_utils import run_bass_kernel_spmd

# dims
B, T, V, D, NH, NKV, L = 2, 2048, 32000, 768, 12, 4, 6
HD, DI, TOPK = 64, 1536, 3
GATE_STRENGTH, EPS = 0.001, 1e-6
P = 128
TC = 512                  # tokens per core
KT = D // P               # 6 d-tiles
DIT = DI // P             # 12 inner tiles
VS = V // 8               # 4000 vocab per core
NVB = 8                   # vocab chunks per core
VB = VS // NVB            # 500
QH = NH // 4              # 3 q heads per core
f32 = mybir.dt.float32
f32r = mybir.dt.float32r
bf16 = mybir.dt.bfloat16

LAST_EXEC_NS = None
_cached = {}


def _build():
    nc = bacc.Bacc(None, target_bir_lowering=False, debug=False, num_devices=8)
    ExT = mybir.ActivationFunctionType

    # ---------------- DRAM I/O ----------------
    idx_t = nc.dram_tensor("idx_t", [P, 4], mybir.dt.int32, kind="ExternalInput")
    tok_emb = nc.dram_tensor("tok_emb", [V, D], f32, kind="ExternalInput")
    qkvw = nc.dram_tensor("qkvw", [L, D, 256], bf16, kind="ExternalInput")
    wvw = nc.dram_tensor("wvw", [L, D, HD], bf16, kind="ExternalInput")
    wow = nc.dram_tensor("wow", [L, 256, D], bf16, kind="ExternalInput")
    pfg = nc.dram_tensor("pfg", [L, D, DI], f32r, kind="ExternalInput")
    pfu = nc.dram_tensor("pfu", [L, D, DI], f32r, kind="ExternalInput")
    pfd = nc.dram_tensor("pfd", [L, DI, D], f32r, kind="ExternalInput")
    eg = nc.dram_tensor("eg", [L, TOPK, D, DI], f32r, kind="ExternalInput")
    eu = nc.dram_tensor("eu", [L, TOPK, D, DI], f32r, kind="ExternalInput")
    edw = nc.dram_tensor("edw", [L, TOPK, DI, D], f32r, kind="ExternalInput")
    twcol = nc.dram_tensor("twcol", [P, KT], f32, kind="ExternalInput")
    embta = nc.dram_tensor("embta", [P, KT, VS], f32r, kind="ExternalInput")
    embtg = nc.dram_tensor("embtg", [P, KT, VS], f32r, kind="ExternalInput")

    la = nc.dram_tensor("la", [8 * TC, VS], f32, kind="ExternalOutput")
    lg = nc.dram_tensor("lg", [8 * TC, VS], f32, kind="ExternalOutput")
    tns = nc.dram_tensor("tns", [L, TC], f32, kind="ExternalOutput")

    # collective bounces
    ht_bounce = nc.dram_tensor("ht_bounce", [P, KT, TC], bf16)
    ht_all_d = nc.dram_tensor("ht_all_d", [4, P, KT, TC], bf16)
    xd_bounce = nc.dram_tensor("xd_bounce", [4, P, KT, TC], bf16)
    xd_own_d = nc.dram_tensor("xd_own_d", [P, KT, TC], bf16)
    xf_bounce = nc.dram_tensor("xf_bounce", [P, KT, TC], f32)
    xfa_d = nc.dram_tensor("xfa_d", [8, P, KT, TC], f32, addr_space="Shared")

    G4 = [[0, 1, 2, 3], [4, 5, 6, 7]]
    G8 = [[0, 1, 2, 3, 4, 5, 6, 7]]

    def wtile(dram2):
        """[p, kt, n] DMA view of a [K, N] dram matrix (K = kt*128+p)."""
        return dram2.rearrange("(kt p) n -> p kt n", p=P)

    with tile.TileContext(nc) as tc:
        import contextlib
        with contextlib.ExitStack() as stack:
            pp = stack.enter_context(tc.tile_pool(name="pp", bufs=1))
            wk = stack.enter_context(tc.tile_pool(name="wk", bufs=1))
            ps = stack.enter_context(tc.tile_pool(name="ps", bufs=1, space="PSUM"))

            # ---- constants
            ones_kf = pp.tile([P, 1], f32, tag="ones_kf")
            nc.vector.memset(ones_kf[:], 1.0)
            ones_kr = pp.tile([P, 1], f32r, tag="ones_kr")
            nc.vector.tensor_copy(out=ones_kr[:], in_=ones_kf[:])
            ones_rf = pp.tile([1, P], f32, tag="ones_rf")
            nc.vector.memset(ones_rf[:], 1.0)
            ones_rr = pp.tile([1, P], f32r, tag="ones_rr")
            nc.vector.tensor_copy(out=ones_rr[:], in_=ones_rf[:])
            ones_rb = pp.tile([1, P], bf16, tag="ones_rb")
            nc.vector.memset(ones_rb[:], 1.0)
            ident = pp.tile([P, P], f32, tag="ident")
            masks.make_identity(nc, ident[:])
            maskT = pp.tile([P, P], bf16, tag="maskT")
            masks.make_upper_triangular(nc, maskT[:], val=1.0, diag=True)
            idx_s = pp.tile([P, 4], mybir.dt.int32, tag="idx_s")
            nc.sync.dma_start(out=idx_s[:], in_=idx_t[:])
            twcol_s = pp.tile([P, KT], f32, tag="twcol_s")
            nc.sync.dma_start(out=twcol_s[:], in_=twcol[:])
            eps_t = pp.tile([1, 1], f32, tag="eps_t")
            nc.vector.memset(eps_t[:], EPS)

            # ---- persistent activations
            xT = pp.tile([P, KT, TC], f32, tag="xT")
            hta = pp.tile([P, KT, 4 * TC], bf16, tag="hta")
            qT_s = pp.tile([64, QH, 4 * TC], bf16, tag="qT_s")
            kT_t = pp.tile([64, 4 * TC], bf16, tag="kT_t")
            v_aug = pp.tile([P, 16, 65], bf16, tag="v_aug")
            nc.vector.memset(v_aug[:, :, 64:65], 1.0)
            oT_s = pp.tile([P, 2, 4 * TC], bf16, tag="oT_s")
            nc.vector.memset(oT_s[64:128, 1, :], 0.0)
            qkvw_s = pp.tile([P, KT, 256], bf16, tag="qkvw_s")
            wvw_s = pp.tile([P, KT, HD], bf16, tag="wvw_s")
            wow_s = pp.tile([P, 2, D], bf16, tag="wow_s")
            xdo_s = pp.tile([P, KT, TC], bf16, tag="xdo_s")

            # ---------- embedding gather + transpose into xT ----------
            for j in range(4):
                xg = wk.tile([P, D], f32, tag="xg", bufs=1)
                nc.gpsimd.indirect_dma_start(
                    out=xg[:], out_offset=None, in_=tok_emb[:],
                    in_offset=bass.IndirectOffsetOnAxis(ap=idx_s[:, j:j + 1], axis=0))
                for k in range(KT):
                    tp = ps.tile([P, P], f32, tag="bc", bufs=2)
                    nc.tensor.transpose(out=tp[:], in_=xg[:, k * P:(k + 1) * P],
                                        identity=ident[:])
                    nc.vector.tensor_copy(out=xT[:, k, j * P:(j + 1) * P], in_=tp[:])

            # ---------- helpers ----------
            def rms_rows(out_ap_fn, bcast_ones, bcast_dtype):
                """out_ap_fn(k) <- xT[:,k,:] * rsqrt(mean_d(x^2)+eps)."""
                ss = ps.tile([1, TC], f32, tag="row", bufs=1, name="ss")
                for k in range(KT):
                    sq = wk.tile([P, TC], f32r, tag="sq", bufs=2, name="sq")
                    nc.vector.tensor_mul(out=sq[:], in0=xT[:, k, :], in1=xT[:, k, :])
                    nc.tensor.matmul(out=ss[:], lhsT=ones_kr[:], rhs=sq[:],
                                     start=(k == 0), stop=(k == KT - 1))
                s_sb = wk.tile([1, TC], f32, tag="s_sb", name="s_sb")
                nc.scalar.activation(out=s_sb[:], in_=ss[:], func=ExT.Sqrt,
                                     scale=1.0 / D, bias=eps_t[:])
                r_row = wk.tile([1, TC], bcast_dtype, tag="r_row", name="r_row")
                with nc.allow_low_precision(reason="rsqrt row broadcast operand"):
                    nc.vector.reciprocal(out=r_row[:], in_=s_sb[:])
                bc = ps.tile([P, TC], f32, tag="bc", bufs=2, name="bc")
                nc.tensor.matmul(out=bc[:], lhsT=bcast_ones[:], rhs=r_row[:],
                                 start=True, stop=True)
                for k in range(KT):
                    nc.vector.tensor_mul(out=out_ap_fn(k), in0=xT[:, k, :], in1=bc[:])

            def ffn(wg_ap, wu_ap, wd_ap, h_in, tension_row=None):
                """x += silu(h@g)*(h@u) @ d on own tokens, fp32r.
                wg_ap/wu_ap: [p, kt, DI] dram views; wd_ap: [p, kt12, D]."""
                hid = wk.tile([P, DIT, TC], f32r, tag="hid", name="hid")
                for half in range(2):
                    wgh = wk.tile([P, KT, DI // 2], f32r, tag="wgh", name="wgh")
                    nc.sync.dma_start(out=wgh[:], in_=wg_ap[:, :, half * 768:half * 768 + 768])
                    wuh = wk.tile([P, KT, DI // 2], f32r, tag="wuh", name="wuh")
                    nc.sync.dma_start(out=wuh[:], in_=wu_ap[:, :, half * 768:half * 768 + 768])
                    for mt2 in range(6):
                        mt = half * 6 + mt2
                        pg = ps.tile([P, TC], f32, tag="mmout", bufs=3, name="pg")
                        for k in range(KT):
                            nc.tensor.matmul(out=pg[:], lhsT=wgh[:, k, mt2 * P:(mt2 + 1) * P],
                                             rhs=h_in[:, k, :], start=(k == 0),
                                             stop=(k == KT - 1))
                        sg = wk.tile([P, TC], f32r, tag="sg", bufs=2, name="sg")
                        nc.scalar.activation(out=sg[:], in_=pg[:], func=ExT.Silu)
                        pu = ps.tile([P, TC], f32, tag="mmout", bufs=3, name="pu")
                        for k in range(KT):
                            nc.tensor.matmul(out=pu[:], lhsT=wuh[:, k, mt2 * P:(mt2 + 1) * P],
                                             rhs=h_in[:, k, :], start=(k == 0),
                                             stop=(k == KT - 1))
                        nc.vector.tensor_mul(out=hid[:, mt, :], in0=sg[:], in1=pu[:])
                tens_sb = None
                if tension_row is not None:
                    tp2 = ps.tile([1, TC], f32, tag="row", bufs=1, name="tp2")
                    for mt in range(DIT):
                        nc.tensor.matmul(out=tp2[:], lhsT=ones_kr[:], rhs=hid[:, mt, :],
                                         start=(mt == 0), stop=(mt == DIT - 1))
                    tens_sb = wk.tile([1, TC], f32r, tag="tens", bufs=2, name="tens_sb")
                    nc.scalar.activation(out=tens_sb[:], in_=tp2[:], func=ExT.Tanh,
                                         scale=1.0 / DI)
                    nc.sync.dma_start(out=tns[tension_row:tension_row + 1, :],
                                      in_=tens_sb[:].bitcast(f32))
                # down: stream N-halves (each half has full K=12)
                for nh in range(2):
                    wdh = wk.tile([P, DIT, D // 2], f32r, tag="wdh", name="wdh")
                    nc.sync.dma_start(out=wdh[:], in_=wd_ap[:, :, nh * 384:nh * 384 + 384])
                    for mt2 in range(3):
                        mt = nh * 3 + mt2
                        pd = ps.tile([P, TC], f32, tag="mmout", bufs=3, name="pd")
                        for k in range(DIT):
                            nc.tensor.matmul(out=pd[:], lhsT=wdh[:, k, mt2 * P:(mt2 + 1) * P],
                                             rhs=hid[:, k, :], start=(k == 0),
                                             stop=(k == DIT - 1))
                        nc.vector.tensor_add(out=xT[:, mt, :], in0=xT[:, mt, :], in1=pd[:])
                return tens_sb

            # ================= layers =================
            tens_prev = None
            for l in range(L):
                # ---- attention ----
                htl = wk.tile([P, KT, TC], bf16, tag="htl", name="htl")
                rms_rows(lambda k: htl[:, k, :], ones_rb, bf16)
                nc.sync.dma_start(out=ht_bounce[:], in_=htl[:])
                nc.gpsimd.collective_compute(
                    "AllGather", mybir.AluOpType.bypass, replica_groups=G4,
                    ins=[ht_bounce[:]], outs=[ht_all_d[:]])
                for b in range(4):
                    nc.sync.dma_start(out=hta[:, :, b * TC:(b + 1) * TC], in_=ht_all_d[b])

                nc.sync.dma_start(out=qkvw_s[:], in_=wtile(qkvw[l]))
                nc.sync.dma_start(out=wvw_s[:], in_=wtile(wvw[l]))
                nc.sync.dma_start(out=wow_s[:], in_=wow[l].rearrange("(kt p) n -> p kt n", p=P))

                # q heads (3x64) | k (64), all at partition base 0
                for i in range(4):
                    for n in range(4):
                        pq = ps.tile([64, TC], f32, tag="mmout", bufs=3, name="pq")
                        for k in range(KT):
                            nc.tensor.matmul(out=pq[:], lhsT=qkvw_s[:, k, 64 * i:64 * (i + 1)],
                                             rhs=hta[:, k, n * TC:(n + 1) * TC],
                                             start=(k == 0), stop=(k == KT - 1))
                        dst = (qT_s[0:64, i, n * TC:(n + 1) * TC] if i < QH
                               else kT_t[0:64, n * TC:(n + 1) * TC])
                        nc.scalar.activation(out=dst, in_=pq[:], func=ExT.Copy)
                # v natural [2048, 64] -> v_aug[:, tt, 0:64]
                for tt in range(16):
                    pv = ps.tile([P, TC], f32, tag="mmout", bufs=3, name="pv")
                    for k in range(KT):
                        nc.tensor.matmul(out=pv[:, 0:HD], lhsT=hta[:, k, tt * P:(tt + 1) * P],
                                         rhs=wvw_s[:, k, :], start=(k == 0),
                                         stop=(k == KT - 1))
                    nc.scalar.activation(out=v_aug[:, tt, 0:64], in_=pv[:, 0:HD], func=ExT.Copy)

                # scores / exp / AV per local head, per query chunk
                for hl in range(QH):
                    qtile, qoff = ((0, 0), (0, 64), (1, 0))[hl]
                    for qc in range(4):
                        oT = ps.tile([65, TC], f32, tag="oT", bufs=2, name="oT")
                        last_kt = 4 * qc + 3
                        for kt in range(last_kt + 1):
                            j = kt - 4 * qc
                            c0 = 128 * j if j > 0 else 0
                            sc = ps.tile([P, TC], f32, tag="mmout", bufs=3, name="sc")
                            nc.tensor.matmul(
                                out=sc[:, c0:TC],
                                lhsT=kT_t[0:64, kt * P:(kt + 1) * P],
                                rhs=qT_s[0:64, hl, qc * TC + c0:(qc + 1) * TC],
                                start=True, stop=True)
                            es = wk.tile([P, TC], bf16, tag="es", bufs=4, name="es")
                            nc.scalar.activation(out=es[:, c0:TC], in_=sc[:, c0:TC],
                                                 func=ExT.Exp)
                            if j >= 0:
                                nc.vector.tensor_mul(out=es[:, c0:c0 + P],
                                                     in0=es[:, c0:c0 + P], in1=maskT[:])
                            nc.tensor.matmul(out=oT[0:65, c0:TC], lhsT=v_aug[:, kt, :],
                                             rhs=es[:, c0:TC], start=(kt == 0),
                                             stop=(kt == last_kt))
                        rec = wk.tile([1, TC], bf16, tag="rec", bufs=2, name="rec")
                        with nc.allow_low_precision(reason="softmax denom broadcast"):
                            nc.vector.reciprocal(out=rec[:], in_=oT[64:65, :])
                        bcd = ps.tile([P, TC], f32, tag="bc", bufs=2, name="bcd")
                        nc.tensor.matmul(out=bcd[0:64, :], lhsT=ones_rb[:, 0:64], rhs=rec[:],
                                         start=True, stop=True)
                        bcs = wk.tile([64, TC], bf16, tag="bcs", bufs=2, name="bcs")
                        nc.scalar.activation(out=bcs[:], in_=bcd[0:64, :], func=ExT.Copy)
                        nc.vector.tensor_mul(
                            out=oT_s[qoff:qoff + 64, qtile, qc * TC:(qc + 1) * TC],
                            in0=oT[0:64, :], in1=bcs[:])

                # wo partial -> xd_bounce blocks, ReduceScatter, add to xT
                for mt in range(KT):
                    for n in range(4):
                        px = ps.tile([P, TC], f32, tag="mmout", bufs=3, name="px")
                        for k in range(2):
                            nc.tensor.matmul(out=px[:], lhsT=wow_s[:, k, mt * P:(mt + 1) * P],
                                             rhs=oT_s[:, k, n * TC:(n + 1) * TC],
                                             start=(k == 0), stop=(k == 1))
                        xdt = wk.tile([P, TC], bf16, tag="xdt", bufs=3, name="xdt")
                        nc.scalar.activation(out=xdt[:], in_=px[:], func=ExT.Copy)
                        nc.sync.dma_start(out=xd_bounce[n, :, mt, :], in_=xdt[:])
                nc.gpsimd.collective_compute(
                    "ReduceScatter", mybir.AluOpType.add, replica_groups=G4,
                    ins=[xd_bounce[:]], outs=[xd_own_d[:]])
                nc.sync.dma_start(out=xdo_s[:], in_=xd_own_d[:])
                for k in range(KT):
                    nc.vector.tensor_add(out=xT[:, k, :], in0=xT[:, k, :], in1=xdo_s[:, k, :])

                # ---- PureField FFN + tension ----
                hpf = wk.tile([P, KT, TC], f32r, tag="hT", name="hpf")
                rms_rows(lambda k: hpf[:, k, :], ones_rr, f32r)
                tens_cur = ffn(wtile(pfg[l]), wtile(pfu[l]), wtile(pfd[l]), hpf,
                               tension_row=l)

                # ---- cs gate (uses previous layer's tension) ----
                if tens_prev is not None:
                    bcc = ps.tile([P, TC], f32, tag="bc", bufs=2, name="bcc")
                    nc.tensor.matmul(out=bcc[:], lhsT=ones_rr[:], rhs=tens_prev[:],
                                     start=True, stop=True)
                    for k in range(KT):
                        cst = wk.tile([P, TC], f32, tag="cst", bufs=2, name="cst")
                        nc.vector.tensor_scalar_mul(cst[:], bcc[:], twcol_s[:, k:k + 1])
                        nc.vector.tensor_add(out=xT[:, k, :], in0=xT[:, k, :], in1=cst[:])
                tens_prev = tens_cur

                # ---- MoE: 3 dense experts on own tokens ----
                hm = wk.tile([P, KT, TC], f32r, tag="hT", name="hm")
                rms_rows(lambda k: hm[:, k, :], ones_rr, f32r)
                for e in range(TOPK):
                    ffn(wtile(eg[l, e]), wtile(eu[l, e]), wtile(edw[l, e]), hm)

            # ---- final norm -> allgather ----
            xf = wk.tile([P, KT, TC], f32r, tag="hT", name="xf")
            rms_rows(lambda k: xf[:, k, :], ones_rr, f32r)
            nc.sync.dma_start(out=xf_bounce[:], in_=xf[:].bitcast(f32))
            nc.gpsimd.collective_compute(
                "AllGather", mybir.AluOpType.bypass, replica_groups=G8,
                ins=[xf_bounce[:]], outs=[xfa_d[:]])

        # ---------- head phase (fresh pools) ----------
        with contextlib.ExitStack() as stack:
            hp = stack.enter_context(tc.tile_pool(name="hp", bufs=1))
            hw = stack.enter_context(tc.tile_pool(name="hw", bufs=3))
            ho = stack.enter_context(tc.tile_pool(name="ho", bufs=4))
            hps = stack.enter_context(tc.tile_pool(name="hps", bufs=4, space="PSUM"))
            xfa_s = hp.tile([P, KT, 8 * TC], f32r, tag="xfa_s")
            for b in range(8):
                nc.sync.dma_start(out=xfa_s[:, :, b * TC:(b + 1) * TC],
                                  in_=xfa_d[b].bitcast(f32r))
            for emb_d, out_d in ((embta, la), (embtg, lg)):
                for nb in range(NVB):
                    wch = hw.tile([P, KT, VB], f32r, tag="wch", name="wch")
                    nc.sync.dma_start(out=wch[:], in_=emb_d[:, :, nb * VB:(nb + 1) * VB])
                    for mt in range(32):
                        pl = hps.tile([P, VB], f32, tag="hmm", name="pl")
                        for k in range(KT):
                            nc.tensor.matmul(out=pl[:], lhsT=xfa_s[:, k, mt * P:(mt + 1) * P],
                                             rhs=wch[:, k, :], start=(k == 0),
                                             stop=(k == KT - 1))
                        ot = ho.tile([P, VB], f32, tag="ot", name="ot")
                        nc.scalar.activation(out=ot[:], in_=pl[:], func=ExT.Copy)
                        nc.sync.dma_start(
                            out=out_d[mt * P:(mt + 1) * P, nb * VB:(nb + 1) * VB],
                            in_=ot[:])

    nc.compile()
    return nc


def _gelu_tanh(x):
    c = np.float32(np.sqrt(2.0 / np.pi))
    return np.float32(0.5) * x * (np.float32(1.0) +
                                  np.tanh(c * (x + np.float32(0.044715) * x * x * x)))


def kernel(**inputs):
    global LAST_EXEC_NS
    f = lambda name: np.asarray(inputs[name], np.float32)
    idx = np.asarray(inputs["idx"]).astype(np.int64)
    cs_states = f("consciousness_states")
    tok_emb = f("tok_emb")
    head_g = f("head_g")
    cv_w1, cv_b1 = f("cv_w1"), f("cv_b1")
    cv_w2, cv_b2 = f("cv_w2"), f("cv_b2")
    tension_w = f("tension_w")
    ln_attn_w, ln_pf_w = f("ln_attn_w"), f("ln_pf_w")
    ln_moe_w, ln_f_w = f("ln_moe_w"), f("ln_f_w")
    wq, wk_, wv, wo = f("wq"), f("wk"), f("wv"), f("wo")
    pf_gate, pf_up, pf_down = f("pf_gate"), f("pf_up"), f("pf_down")
    e_gate, e_up, e_down = f("e_gate"), f("e_up"), f("e_down")
    router_w, router_b = f("router_w"), f("router_b")

    # ---- host routing (tiny; depends only on consciousness_states) ----
    pooled = cs_states.mean(axis=1)
    h1 = _gelu_tanh(pooled @ cv_w1 + cv_b1)
    cv = (1.0 / (1.0 + np.exp(-(h1 @ cv_w2 + cv_b2)))).astype(np.float32)   # (B,10)
    ti = np.zeros((L, B, TOPK), np.int64)
    tw = np.zeros((L, B, TOPK), np.float32)
    for l in range(L):
        logits = cv @ router_w[l] + router_b[l]
        e = np.exp(logits - logits.max(axis=-1, keepdims=True))
        rw = (e / e.sum(axis=-1, keepdims=True)).astype(np.float32)
        order = np.argsort(-rw, axis=-1, kind="stable")[:, :TOPK]
        vals = np.take_along_axis(rw, order, axis=-1)
        ti[l] = order
        tw[l] = vals / vals.sum(axis=-1, keepdims=True)

    import ml_dtypes
    bfl = ml_dtypes.bfloat16

    # ---- fold layernorm gains / scales into weights ----
    wq_f = (ln_attn_w[:, :, None] * wq) * np.float32(1.0 / np.sqrt(HD))
    wk_f = ln_attn_w[:, :, None] * wk_
    wv_f = ln_attn_w[:, :, None] * wv
    pfg_f = np.ascontiguousarray(ln_pf_w[:, :, None] * pf_gate)
    pfu_f = np.ascontiguousarray(ln_pf_w[:, :, None] * pf_up)
    pfd_f = np.ascontiguousarray(pf_down)
    emb_a_eff = tok_emb * ln_f_w[None, :]
    emb_g_eff = head_g * ln_f_w[None, :]

    if "nc" not in _cached:
        _cached["nc"] = _build()
    nc = _cached["nc"]

    def embt_slice(emb_eff, c):
        sl = emb_eff[c * VS:(c + 1) * VS].T          # [768, VS]
        return np.ascontiguousarray(sl.reshape(KT, P, VS).transpose(1, 0, 2))

    twcol_np = np.ascontiguousarray(
        (tension_w * np.float32(GATE_STRENGTH)).reshape(KT, P).T)

    lidx = np.arange(L)[:, None]
    in_maps = []
    for c in range(8):
        g, m = c // 4, c % 4
        idx_loc = idx[g, m * TC:(m + 1) * TC].astype(np.int32)
        idx_tile = np.ascontiguousarray(idx_loc.reshape(4, P).T)   # [128,4]
        qkv_loc = np.concatenate(
            [wq_f[:, :, 192 * m:192 * (m + 1)], wk_f[:, :, HD * m:HD * (m + 1)]],
            axis=2)                                                # [L,768,256]
        wv_loc = np.ascontiguousarray(wv_f[:, :, HD * m:HD * (m + 1)])
        wo_loc = np.zeros((L, 256, D), np.float32)
        wo_loc[:, 0:192, :] = wo[:, 192 * m:192 * (m + 1), :]
        eg_loc = np.ascontiguousarray(ln_moe_w[:, None, :, None] * e_gate[lidx, ti[:, g]])
        eu_loc = np.ascontiguousarray(ln_moe_w[:, None, :, None] * e_up[lidx, ti[:, g]])
        ed_loc = np.ascontiguousarray(tw[:, g, :, None, None] * e_down[lidx, ti[:, g]])
        in_maps.append(dict(
            idx_t=idx_tile,
            tok_emb=tok_emb,
            qkvw=qkv_loc.astype(bfl),
            wvw=wv_loc.astype(bfl),
            wow=wo_loc.astype(bfl),
            pfg=pfg_f, pfu=pfu_f, pfd=pfd_f,
            eg=eg_loc, eu=eu_loc, edw=ed_loc,
            twcol=twcol_np,
            embta=embt_slice(emb_a_eff, c),
            embtg=embt_slice(emb_g_eff, c),
        ))

    res = run_bass_kernel_spmd(nc, in_maps, list(range(8)))
    LAST_EXEC_NS = res.exec_time_ns

    logits_a = np.concatenate([res.results[c]["la"] for c in range(8)],
                              axis=1).reshape(B, T, V)
    logits_g = np.concatenate([res.results[c]["lg"] for c in range(8)],
                              axis=1).reshape(B, T, V)
    tensions = np.zeros((L, B, T), np.float32)
    for c in range(8):
        g, m = c // 4, c % 4
        tensions[:, g, m * TC:(m + 1) * TC] = res.results[c]["tns"]
    return logits_a, logits_g, tensions
